# revision 4
# baseline (speedup 1.0000x reference)
"""Causal self-attention Trainium2 kernel (8 NeuronCores) — v3.

Sharding: core = (batch b, head-group hg); each core owns 4 of the 16 heads
(256 of 1024 q/k/v dims) for one batch element. W_o is row-parallel; host
sums the 4 partial outputs (+ b_o).

v3 changes vs v2 (172-204us):
  - Q/K projections run as fp8e4 DoubleRow matmuls: contraction-chunk PAIRS
    (256-deep) per instruction at the same per-column rate as bf16 (HW
    microbench: 218ns for [128,2,128]x[128,2,512] — true 2x).  x and W_q/W_k
    are pre-scaled on host (x*8, W*256 resp *2048) so fp8e4 sees its full
    range; the 2^28 product scale is folded into the exp() activation scale.
    V/W_o stay bf16 (fp8 there fails the 2e-2 accuracy gate; measured by
    numpy simulation of e4m3 quantization).
  - Attention processes key-chunk PAIRS per round: one 4-bank PSUM score
    tile [128, 2heads*2chunks, 512] and ONE wide exp ACTIVATE per pair
    (halves ACT instruction count; ACT is fixed 1 elem/cycle/lane so only
    the per-instruction ~300ns overhead is saved).
  - Causal masks shrunk to the 128-wide triangular boundary chunk only
    (off-boundary columns of a diagonal chunk are fully valid; the unread
    gap region of the second chunk in a pair is never consumed).
  - Softmax-normalize in-place multiplies moved to gpsimd (SBUF<->SBUF);
    DVE keeps the PSUM-reading copies/fused ops.
"""

import sys

for _p in ("/opt/trn_rl_repo",):
    if _p not in sys.path:
        sys.path.insert(0, _p)

import numpy as np
import ml_dtypes

import concourse.bass as bass
import concourse.bacc as bacc
import concourse.mybir as mybir
from concourse import tile
from concourse.bass_utils import run_bass_kernel_spmd

P = 128
S = 2048
D = 1024
DG = 256
DH = 64
NHG = 4
EC = D // P
ECP = EC // 2
KC = S // P
QTW = 512
NQT = S // QTW
F32 = mybir.dt.float32
BF16 = mybir.dt.bfloat16
F8 = mybir.dt.float8e4
DR = mybir.MatmulPerfMode.DoubleRow
AF = mybir.ActivationFunctionType

# host-side scales for the fp8 Q/K projection path
XS = 8.0          # x -> fp8 scale
WQS = 256.0       # W_q -> fp8 scale (2048 with the 1/sqrt(dh)=1/8 folded in)
WKS = 2048.0      # W_k -> fp8 scale
EXPS = 1.0 / (XS * WQS * XS * WKS * 8.0)  # de-scale (incl 1/sqrt(dh)) in exp()

LAST_EXEC_NS = None
LAST_RESULTS = None


def _emit(tc, aps):
    nc = tc.nc
    xt8_d, xtv_d, wq8_d, wk8_d, wvt_d, wot_d, bq_d, bk_d, bv_d, yt_d, yt2_d = aps

    with (
        tc.tile_pool(name="const", bufs=1) as constp,
        tc.tile_pool(name="wpool", bufs=1) as wp,
        tc.tile_pool(name="xpool", bufs=1) as xp,
        tc.tile_pool(name="qkvp", bufs=1) as qkvp,
        tc.tile_pool(name="outp", bufs=1) as outp,
        tc.tile_pool(name="aep", bufs=4) as aep,
        tc.tile_pool(name="normp", bufs=2) as normp,
        tc.tile_pool(name="sgp", bufs=4) as sgp,
        tc.tile_pool(name="psp", bufs=1, space="PSUM") as psp,
    ):
        # ---- persistent SBUF tensors ----
        warm_sb = constp.tile([P, QTW], BF16, name="warm_sb")
        maskm = constp.tile([P, P], BF16, name="maskm")
        bq_sb = constp.tile([P, 2], F32, name="bq_sb")
        bk_sb = constp.tile([P, 2], F32, name="bk_sb")
        bv1_sb = constp.tile([1, DG], F32, name="bv1_sb")
        bvb_sb = constp.tile([P, DG], F32, name="bvb_sb")

        wq8_sb = wp.tile([P, EC, DG], F8, name="wq8_sb")
        wk8_sb = wp.tile([P, EC, DG], F8, name="wk8_sb")
        wvt_sb = wp.tile([P, EC, DG], BF16, name="wvt_sb")
        wot_sb = wp.tile([P, 2, D], BF16, name="wot_sb")
        # odd-half W_o rows re-based at partition 0 for the tail (bass needs
        # matching base partitions between stationary and moving operands)
        wot_lo = wp.tile([DH, 2, D], BF16, name="wot_lo")

        xt8_sb = xp.tile([P, EC, NQT, QTW], F8, name="xt8_sb")
        xtv_sb = xp.tile([P, EC, NQT, QTW], BF16, name="xtv_sb")

        qt_sb = qkvp.tile([P, 2, S], BF16, name="qt_sb")
        kt_sb = qkvp.tile([P, 2, S], BF16, name="kt_sb")
        v_sb = qkvp.tile([P, KC, NHG, DH + 1], BF16, name="v_sb")
        outt_sb = outp.tile([P, 2, S], BF16, name="outt_sb")

        # ---- input DMAs (few, large; the tile-0 pieces first so the tile-0
        # Q/K projections and first V chunks start sooner) ----
        xt8_r = xt8_d.rearrange("(e p) (t q) -> p e t q", p=P, q=QTW)
        xtv_r = xtv_d.rearrange("(e p) (t q) -> p e t q", p=P, q=QTW)
        yt_r = yt_d.rearrange("(a p) (t q) -> p a t q", p=P, q=QTW)
        yt2_r = yt2_d.rearrange("(a p) q -> p a q", p=P)
        # two concurrent input queues: the fp8 Q/K stream (sync) feeds the
        # prologue projections; the bf16 V stream (vector hwdge) runs beside
        nc.sync.dma_start(wq8_sb[:, :, :], wq8_d.rearrange("(e p) g -> p e g", p=P))
        nc.sync.dma_start(xt8_sb[:, :, 0, :], xt8_r[:, :, 0, :])
        nc.sync.dma_start(wk8_sb[:, :, :], wk8_d.rearrange("(e p) g -> p e g", p=P))
        for c_ in range(2):
            nc.sync.dma_start(bq_sb[:, c_ : c_ + 1], bq_d[c_ * P : (c_ + 1) * P, :])
            nc.sync.dma_start(bk_sb[:, c_ : c_ + 1], bk_d[c_ * P : (c_ + 1) * P, :])
        nc.vector.dma_start(wvt_sb[:, :, :], wvt_d.rearrange("(e p) g -> p e g", p=P))
        nc.vector.dma_start(bv1_sb[:, :], bv_d[:, :])
        nc.vector.dma_start(xtv_sb[:, :, 0, :], xtv_r[:, :, 0, :])
        nc.sync.dma_start(xt8_sb[:, :, 1, :], xt8_r[:, :, 1, :])
        nc.vector.dma_start(xtv_sb[:, :, 1, :], xtv_r[:, :, 1, :])
        nc.sync.dma_start(xt8_sb[:, :, 2, :], xt8_r[:, :, 2, :])
        nc.sync.dma_start(xt8_sb[:, :, 3, :], xt8_r[:, :, 3, :])
        nc.vector.dma_start(xtv_sb[:, :, 2, :], xtv_r[:, :, 2, :])
        nc.vector.dma_start(xtv_sb[:, :, 3, :], xtv_r[:, :, 3, :])
        nc.sync.dma_start(wot_sb[:, :, :], wot_d.rearrange("(a p) m -> p a m", p=P))
        nc.sync.dma_start(
            wot_lo[:, :, :],
            wot_d.rearrange("(a q p) m -> p a q m", p=DH, q=2)[:, :, 1, :],
        )

        # ---- PE warm-up: throwaway matmuls on a dep-free const tile during
        # the input DMA wait so the projections start at full clock ----
        nc.gpsimd.memset(warm_sb[:, :], 0.0)
        trash = psp.tile([DH + 1, QTW], F32, name="trash", tag="av", bufs=2)
        for _ in range(8):
            nc.tensor.matmul(
                trash[:, :],
                warm_sb[:, 0 : DH + 1],
                warm_sb[:, :],
                start=True,
                stop=True,
            )

        # ---- constants ----
        nc.gpsimd.partition_broadcast(bvb_sb[:, :], bv1_sb[:, :], channels=P)
        # causal multiply-mask for the 128-wide triangular boundary chunk:
        # maskm[x, i] = 1 if i >= x else 0
        nc.gpsimd.memset(maskm[:, :], 1.0)
        nc.gpsimd.affine_select(
            out=maskm[:, :],
            in_=maskm[:, :],
            compare_op=mybir.AluOpType.is_ge,
            fill=0.0,
            base=0,
            pattern=[[1, P]],
            channel_multiplier=-1,
        )
        # ones column in V: A@V also produces the softmax denominator
        nc.vector.memset(v_sb[:, :, :, DH : DH + 1], 1.0)

        # ---- filler closures (PE work interleaved into attention rounds) ----
        def qk_closures(qk, t):
            w8, dst_sb, b_sb = (
                (wq8_sb, qt_sb, bq_sb) if qk == "q" else (wk8_sb, kt_sb, bk_sb)
            )

            def mk(db):
                def c():
                    ps = psp.tile(
                        [P, QTW], F32, name=f"ps_{qk}{t}{db}", tag="fil", bufs=2
                    )
                    for e in range(ECP):
                        nc.tensor.matmul(
                            ps[:, :],
                            w8[:, 2 * e : 2 * e + 2, db * P : (db + 1) * P],
                            xt8_sb[:, 2 * e : 2 * e + 2, t, :],
                            start=(e == 0),
                            stop=(e == ECP - 1),
                            perf_mode=DR,
                        )
                    nc.vector.tensor_scalar_add(
                        dst_sb[:, db, t * QTW : (t + 1) * QTW],
                        ps[:, :],
                        b_sb[:, db : db + 1],
                    )

                return c

            return [mk(0), mk(1)]

        def v_closure(sc):
            def c1():
                pv = psp.tile([P, DG], F32, name=f"pv{sc}", tag="fil", bufs=2)
                tb, i = divmod(sc, 4)
                for ec in range(EC):
                    nc.tensor.matmul(
                        pv[:, :],
                        xtv_sb[:, ec, tb, i * P : (i + 1) * P],
                        wvt_sb[:, ec, :],
                        start=(ec == 0),
                        stop=(ec == EC - 1),
                    )
                nc.vector.tensor_add(
                    v_sb[:, sc, :, 0:DH],
                    pv[:, :].rearrange("p (h d) -> p h d", h=NHG),
                    bvb_sb[:, :].rearrange("p (h d) -> p h d", h=NHG),
                )

            return c1

        odd3 = {}  # dc -> staged odd-half norm tile of the last q-tile

        def wo_closures(t, po=None, accum=False):
            # po=None: full-contraction WO for tile t.  po=0/1: half
            # contraction over that head-pair's rows only (used to split the
            # last tile's WO around the final norm; the po=1 half goes to a
            # separate yt2 output that the host adds in).
            cls = []
            for mc in range(8):

                def c(mc=mc):
                    py = psp.tile([P, QTW], F32, name=f"py{t}_{mc}", tag="fil", bufs=2)
                    r0 = 0 if po in (None, 0) else DH
                    r1 = P if po in (None, 1) else DH
                    for dcw in range(2):
                        if accum:
                            # read the staged odd-norm tiles (partitions 0-63)
                            # against the re-based odd W_o rows.  Skips the
                            # SBUF->SBUF outt DMA on the tail critical path.
                            sta = wot_lo[:, dcw, mc * P : (mc + 1) * P]
                            mov = odd3[dcw]
                        else:
                            sta = wot_sb[r0:r1, dcw, mc * P : (mc + 1) * P]
                            mov = outt_sb[r0:r1, dcw, t * QTW : (t + 1) * QTW]
                        nc.tensor.matmul(
                            py[:, :],
                            sta,
                            mov,
                            start=(dcw == 0),
                            stop=(dcw == 1),
                        )
                    sg = sgp.tile([P, QTW], BF16, name=f"sg{t}_{mc}", tag="sg")
                    # gpsimd has no PSUM port; split staging copies DVE/ACT.
                    # Tail half: copies on ACT (DVE runs the final norms),
                    # DMAs issued from the ACT hwdge queue.
                    if mc % 2 == 0 and not accum:
                        nc.vector.tensor_copy(sg[:, :], py[:, :])
                    else:
                        nc.scalar.copy(sg[:, :], py[:, :])
                    if accum:
                        nc.scalar.dma_start(yt2_r[:, mc, :], sg[:, :])
                    else:
                        nc.sync.dma_start(yt_r[:, mc, t, :], sg[:, :])

                cls.append(c)
            return cls

        # ---- attention pass for (q-tile t, head pair m); key chunks are
        # processed in PAIRS: one 4-bank PSUM score tile and one wide exp ----
        def pass_tm(t, m, flist):
            heads = (m, m + 2)
            po = m
            cmax = 4 * t + 4
            npair = cmax // 2

            avs = {
                h: psp.tile([DH + 1, QTW], F32, name=f"av{t}{m}{h}", tag="av", bufs=2)
                for h in heads
            }

            def emit_pair(r):
                c0 = 2 * r
                j0 = c0 - 4 * t
                q0A = P * j0 if j0 >= 0 else 0
                stp = psp.tile(
                    [P, 4, QTW], F32, name=f"st{t}{m}{r}", tag="st", bufs=1
                )
                for i, h in enumerate(heads):
                    dc = h // 2
                    for jj in range(2):
                        c = c0 + jj
                        j = c - 4 * t
                        q0 = P * j if j >= 0 else 0
                        nc.tensor.matmul(
                            stp[:, 2 * i + jj, q0:QTW],
                            kt_sb[po * DH : (po + 1) * DH, dc, c * P : (c + 1) * P],
                            qt_sb[
                                po * DH : (po + 1) * DH,
                                dc,
                                t * QTW + q0 : (t + 1) * QTW,
                            ],
                            start=True,
                            stop=True,
                        )
                ae = aep.tile([P, 4, QTW], BF16, name=f"ae{t}{m}{r}", tag="ae")
                # scores are exp'd straight out of PSUM in ONE wide activation
                # per chunk-pair; the fp8 Q/K product scale is de-applied here
                nc.scalar.activation(
                    ae[:, :, q0A:QTW], stp[:, :, q0A:QTW], AF.Exp, scale=EXPS
                )
                if j0 >= 0:  # diagonal pair: 128-wide triangular masks only
                    for i in range(2):
                        for jj in range(2):
                            q0 = P * (j0 + jj)
                            nc.vector.tensor_mul(
                                ae[:, 2 * i + jj, q0 : q0 + P],
                                ae[:, 2 * i + jj, q0 : q0 + P],
                                maskm[:, :],
                            )
                return ae, c0

            def emit_av(ae, c0):
                for i, h in enumerate(heads):
                    for jj in range(2):
                        c = c0 + jj
                        j = c - 4 * t
                        q0 = P * j if j >= 0 else 0
                        nc.tensor.matmul(
                            avs[h][:, q0:QTW],
                            v_sb[:, c, h, :],
                            ae[:, 2 * i + jj, q0:QTW],
                            start=(c == 0),
                            stop=(c == cmax - 1),
                        )

            prev = None
            for r in range(npair):
                cur = emit_pair(r)
                if prev is not None:
                    emit_av(*prev)
                # fillers last: keeps the diag-pair mask at the head of the
                # in-order DVE stream while giving the PE overflow work to
                # cover the exp latency (stp has a single PSUM buffer)
                npop = -(-len(flist) // (npair - r))
                for _ in range(npop):
                    flist.pop(0)()
                prev = cur
            emit_av(*prev)

            # normalize: row DH of av is the softmax denominator.  All
            # vector-op operands must share a start partition (BIR verifier),
            # so odd head pairs (po=1) stage in partitions 0-63 and DMA into
            # outt rows 64-127.
            for i, h in enumerate(heads):
                dc = h // 2
                av = avs[h]
                # hw InstReciprocal is ~6 cycles/elem of microcode; the
                # approx-fast custom op is ~5x cheaper and plenty accurate.
                # Stage the denominator at partition 0 first (plain copies may
                # cross partition starts; 2-operand ops may not).
                den = normp.tile([1, QTW], F32, name=f"den{t}{m}{h}", tag="den")
                nc.vector.tensor_copy(den[:, :], av[DH : DH + 1, :])
                rec = normp.tile([1, QTW], F32, name=f"rec{t}{m}{h}", tag="rec")
                nc.vector.reciprocal_approx_fast(rec[:, :], den[:, :])
                bc = normp.tile([DH, QTW], F32, name=f"bc{t}{m}{h}", tag="bc")
                nc.gpsimd.partition_broadcast(bc[:, :], rec[:, :], channels=DH)
                if po == 0:
                    dst = outt_sb[0:DH, dc, t * QTW : (t + 1) * QTW]
                else:
                    dst = normp.tile([DH, QTW], BF16, name=f"odd{t}{m}{h}", tag="odd")
                    dst = dst[:, :]
                if i == 0:
                    # copy first (frees the av PSUM slot without waiting on
                    # the broadcast); the SBUF-only in-place normalize runs
                    # on gpsimd right behind its own broadcast
                    nc.vector.tensor_copy(dst, av[0:DH, :])
                    nc.gpsimd.tensor_mul(dst, dst, bc[:, :])
                else:
                    # fused copy*norm; this av slot isn't needed again as soon
                    nc.vector.scalar_tensor_tensor(
                        dst,
                        av[0:DH, :],
                        1.0,
                        bc[:, :],
                        mybir.AluOpType.mult,
                        mybir.AluOpType.mult,
                    )
                if po == 1:
                    if t == NQT - 1:
                        # last tile: the odd half feeds only the tail W_o,
                        # which reads the staged tile directly
                        odd3[dc] = dst
                    else:
                        nc.sync.dma_start(
                            outt_sb[DH:P, dc, t * QTW : (t + 1) * QTW], dst
                        )

        # ---- prologue: tile-0 Q/K projection + first V chunks ----
        for cl in qk_closures("q", 0):
            cl()
        for cl in qk_closures("k", 0):
            cl()
        v_closure(0)()
        v_closure(1)()

        # ---- static filler plan per pass ----
        q1, k1 = qk_closures("q", 1), qk_closures("k", 1)
        q2, k2 = qk_closures("q", 2), qk_closures("k", 2)
        q3, k3 = qk_closures("q", 3), qk_closures("k", 3)
        plan = {
            (0, 0): [v_closure(2), v_closure(3)] + q1,
            (0, 1): k1 + [v_closure(4), v_closure(5)],
            (1, 0): [v_closure(6), v_closure(7)] + q2 + k2 + [v_closure(8), v_closure(9)],
            (1, 1): [v_closure(10), v_closure(11)] + wo_closures(0),
            (2, 0): [v_closure(12), v_closure(13), v_closure(14), v_closure(15)] + q3 + k3,
            (2, 1): wo_closures(1),
            (3, 0): wo_closures(2),
            # last tile: po=0 WO half only needs pass (3,0) results, so it
            # overlaps the final pass; the po=1 half + accum-DMA is the tail
            (3, 1): wo_closures(3, po=0),
        }
        for t in range(NQT):
            for m in range(2):
                pass_tm(t, m, plan[(t, m)])
        for cl in wo_closures(3, po=1, accum=True):
            cl()


_NC_CACHE = None


def build_nc():
    global _NC_CACHE
    if _NC_CACHE is not None:
        return _NC_CACHE
    nc = bacc.Bacc("TRN2")
    xt8 = nc.dram_tensor("xt8", [D, S], F8, kind="ExternalInput")
    xtv = nc.dram_tensor("xtv", [D, S], BF16, kind="ExternalInput")
    wq8 = nc.dram_tensor("wq8", [D, DG], F8, kind="ExternalInput")
    wk8 = nc.dram_tensor("wk8", [D, DG], F8, kind="ExternalInput")
    wvt = nc.dram_tensor("wvt", [D, DG], BF16, kind="ExternalInput")
    wot = nc.dram_tensor("wot", [DG, D], BF16, kind="ExternalInput")
    bq = nc.dram_tensor("bq", [DG, 1], F32, kind="ExternalInput")
    bk = nc.dram_tensor("bk", [DG, 1], F32, kind="ExternalInput")
    bv = nc.dram_tensor("bv", [1, DG], F32, kind="ExternalInput")
    yt = nc.dram_tensor("yt", [D, S], BF16, kind="ExternalOutput")
    yt2 = nc.dram_tensor("yt2", [D, QTW], BF16, kind="ExternalOutput")
    aps = tuple(
        h.ap() for h in (xt8, xtv, wq8, wk8, wvt, wot, bq, bk, bv, yt, yt2)
    )
    with tile.TileContext(nc) as tc:
        _emit(tc, aps)
    nc.finalize()
    _NC_CACHE = nc
    return nc


def make_in_maps(x, W_q, b_q, W_k, b_k, W_v, b_v, W_o):
    bf = ml_dtypes.bfloat16
    f8 = ml_dtypes.float8_e4m3
    in_maps = []
    xts = [np.asarray(x)[b].T for b in range(2)]
    xt8s = [
        np.ascontiguousarray(np.clip(xt * XS, -240, 240).astype(f8)) for xt in xts
    ]
    xtvs = [np.ascontiguousarray(xt.astype(bf)) for xt in xts]
    for core in range(8):
        b, hg = divmod(core, 4)
        sl = slice(hg * DG, (hg + 1) * DG)
        in_maps.append(
            {
                "xt8": xt8s[b],
                "xtv": xtvs[b],
                # W_q carries the 1/sqrt(dh) fold; fp8 scales de-applied in exp
                "wq8": np.ascontiguousarray(
                    np.clip(np.asarray(W_q)[sl, :] * WQS, -240, 240).T.astype(f8)
                ),
                "wk8": np.ascontiguousarray(
                    np.clip(np.asarray(W_k)[sl, :] * WKS, -240, 240).T.astype(f8)
                ),
                "wvt": np.ascontiguousarray(np.asarray(W_v)[sl, :].T.astype(bf)),
                "wot": np.ascontiguousarray(np.asarray(W_o)[:, sl].T.astype(bf)),
                "bq": np.ascontiguousarray(
                    (np.asarray(b_q)[sl] * (XS * WQS)).reshape(DG, 1),
                    dtype=np.float32,
                ),
                "bk": np.ascontiguousarray(
                    (np.asarray(b_k)[sl] * (XS * WKS)).reshape(DG, 1),
                    dtype=np.float32,
                ),
                "bv": np.ascontiguousarray(
                    np.asarray(b_v)[sl].reshape(1, DG), dtype=np.float32
                ),
            }
        )
    return in_maps


def kernel(x, W_q, b_q, W_k, b_k, W_v, b_v, W_o, b_o, _trace=False):
    global LAST_EXEC_NS, LAST_RESULTS
    nc = build_nc()
    in_maps = make_in_maps(x, W_q, b_q, W_k, b_k, W_v, b_v, W_o)
    kw = {"trace": True} if _trace else {}
    res = run_bass_kernel_spmd(nc, in_maps, core_ids=list(range(8)), **kw)
    LAST_EXEC_NS = res.exec_time_ns
    LAST_RESULTS = res
    b_o = np.asarray(b_o, dtype=np.float32)
    out = np.empty((2, S, D), np.float32)
    for b in range(2):
        ysum = np.zeros((D, S), np.float32)
        for hg in range(4):
            r = res.results[4 * b + hg]
            ysum += r["yt"].astype(np.float32)
            # last q-tile's odd-head W_o half is shipped separately (avoids a
            # read-modify-write DRAM accumulate on the critical tail)
            ysum[:, 3 * QTW :] += r["yt2"].astype(np.float32)
        out[b] = ysum.T + b_o
    return out
